# revision 8
# baseline (speedup 1.0000x reference)
"""Trainium2 Bass kernel for nn_DeConvAfterDownSampling.

Math (from the reference): with s[n] = sum_w x[b,c,h,w] flattened over
n = (b,c,h), Wf = W.reshape(F, P):

    out[0, f, n, p] = relu(s[n] * Wf[f, p] + b[f])      # (1, F, N, P)

N = 8*64*64 = 32768, F = 64, P = 25.  Output is ~210 MB fp32 while inputs
are ~8 MB, so the kernel is bound by the output HBM write.

Sharding: data-parallel over n across 8 cores (N_LOCAL = 4096 per core);
W and b replicated; no cross-core communication.

Per-core plan (partitions = (h, f) with h in {0,1} stacking two n-halves
so all 128 partitions are used; n is tiled as [256, 256, 512 x 7] — small
first tiles shorten the pipeline-fill ramp):
  1. Per-tile DMAs load x chunks in natural row order (partition q holds
     rpp = tn/128 consecutive rows -> rpp*256 B contiguous per partition,
     full DMA rate).
  2. PE transposes each (128 q, 64 w) row-group c -> (64 w, 128 j) in
     PSUM (column j <-> n = n0 + j*rpp + c); VectorE copies split the
     columns into partition halves, assembling xT (128=(h,w), rpp, 64).
  3. One K=128 matmul with a constant block-diagonal ones matrix E
     (E[(h',w),(h,f)] = (h==h')) reduces over w AND broadcasts across all
     f at once; its rhs AP streams (j', c) so the PSUM free axis comes
     out in exact n order: s_bcast[(h,f), m] = s[n0 + (tn/2)h + m].
  4. For each p in 0..24 one elementwise op computes
     relu(W[f,p] * s + b[f]) with W[:,p] as per-partition scale (and, when
     b != 0, b as per-partition bias), writing the (stride 25) p-slice of
     the output tile.  Ops are split across ScalarE (activation, reads
     s from PSUM), GpSimd and VectorE (tensor_scalar mult + max 0, read
     an SBUF copy of s).
  5. One 1.6-3.3 MB DMA per tile writes the (128, tn/2, 25) tile to HBM;
     the per-partition free layout (n-major, p-minor) is exactly
     contiguous HBM order, so each partition is one contiguous chunk.

TimelineSim cost model: 82.6 us per core (output-DMA bound: 26.2 MB fp32
written per core at ~360 GB/s = 72.8 us with zero inter-DMA gaps, 8.4 us
pipeline-fill ramp, 1.4 us drain).  Rel err vs the jax reference: 3.3e-07.
"""

import numpy as np

import concourse.bass as bass
import concourse.mybir as mybir
from concourse import bacc, masks, tile
from concourse.bass_utils import run_bass_kernel_spmd
from concourse.tile_rust import add_dep_helper

F32 = mybir.dt.float32
F16 = mybir.dt.float16

N_CORES = 8
B, C, H, WDIM = 8, 64, 64, 64
F, P = 64, 25
N_TOTAL = B * C * H          # 32768
N_LOCAL = N_TOTAL // N_CORES  # 4096
# Per-tile n sizes: small first tiles shorten the pipeline-fill ramp;
# large tail tiles make the output DMAs 6.5 MB for real-HW DMA efficiency
# (descriptor overhead amortizes with transfer size).
TILE_SIZES = [256, 256, 512, 512, 512, 1024, 1024]
assert sum(TILE_SIZES) == N_LOCAL
NPART = 128

# Engine split for the 25 per-p elementwise ops (b == 0 fast path):
# ScalarE activation / GpSimd tensor_scalar / VectorE tensor_scalar.
# Small (ramp) tiles bias away from ScalarE, whose sequencer is slow.
def engine_split(tn):
    if tn <= 128:
        return set(range(3)), set(range(3, 10))
    if tn <= 256:
        return set(range(4)), set(range(4, 11))
    if tn == 512:
        return set(range(6)), set(range(6, 13))
    return set(range(8)), set(range(8, 14))


def build_bass(
    with_bias: bool,
    repeat: int = 1,
    skip_out_dma: bool = False,
    out_f32: bool = False,
    split: str = "act",  # "act" | "mixed" | "dve" | "actdve"
    out_dt=None,
) -> bass.Bass:
    """repeat > 1 re-executes the whole per-tile pipeline that many times
    (identical work, same output writes) — used only by the timing harness
    to make device time resolvable above the per-dispatch proxy overhead."""
    nc = bacc.Bacc(None)

    x_d = nc.dram_tensor("x", (N_LOCAL, WDIM), F32, kind="ExternalInput")
    w_d = nc.dram_tensor("W", (F, P), F32, kind="ExternalInput")
    b_d = nc.dram_tensor("b", (F, 1), F32, kind="ExternalInput")
    o_d = nc.dram_tensor("out", (F, N_LOCAL, P), F32, kind="ExternalOutput")

    with tile.TileContext(nc) as tc:
        with (
            tc.tile_pool(name="const", bufs=1) as constp,
            tc.tile_pool(name="xin", bufs=1) as xinp,
            tc.tile_pool(name="work", bufs=3) as workp,
            tc.tile_pool(name="outp", bufs=3) as outp,
            tc.tile_pool(name="psum", bufs=2, space="PSUM") as psump,
            tc.tile_pool(name="psum3", bufs=3, space="PSUM") as psump3,
        ):
            # Pool builds identity + E first: they gate the PE transposes.
            ident = constp.tile([NPART, NPART], F32)
            masks.make_identity(nc, ident[:])

            # Block-diagonal ones: E[k, i] = 1 iff k//64 == i//64.
            e_mat = constp.tile([NPART, NPART], F32)
            nc.gpsimd.memset(e_mat[:], 0.0)
            nc.gpsimd.memset(e_mat[0:64, 0:64], 1.0)
            nc.gpsimd.memset(e_mat[64:128, 64:128], 1.0)

            # Pull the ACT table load (~1.3 us) off the critical path: a
            # dummy Relu at t=0 makes insert_act_table_loads put it first.
            warm = constp.tile([NPART, 1], F32)
            nc.vector.memset(warm[:], 0.0)
            warm_out = constp.tile([NPART, 1], F32)
            nc.scalar.activation(
                warm_out[:], warm[:], mybir.ActivationFunctionType.Relu
            )

            # W columns replicated on both partition halves, via the ACT
            # HWDGE ring so neither the x loads (SP ring) nor the Pool
            # engine (identity/E) are delayed.
            wcols = constp.tile([NPART, P], F32)
            nc.scalar.dma_start(wcols[0:64, :], w_d[:, :])
            nc.scalar.dma_start(wcols[64:128, :], w_d[:, :])
            if with_bias:
                bcol = constp.tile([NPART, 1], F32)
                nc.scalar.dma_start(bcol[0:64, :], b_d[:, :])
                nc.scalar.dma_start(bcol[64:128, :], b_d[:, :])
                bias_arg = bcol[:, 0:1]
            else:
                # b is all zeros: skip the load, use an immediate bias.
                nc.gpsimd.dma_start(constp.tile([1, 1], F32, name="bjunk")[:],
                                    b_d[0:1, :])  # keep "b" a live input
                bias_arg = 0.0

            tile_offsets = [sum(TILE_SIZES[:u]) for u in range(len(TILE_SIZES))]

            # --- load x (critical path), one chunk per tile ---
            # Natural row order: partition q holds rpp = tn/128 consecutive
            # rows (rpp*256 B contiguous per partition -> full DMA rate).
            # n = n0 + q*rpp + c.
            x_chunks = []
            for u, (n0, tn) in enumerate(zip(tile_offsets, TILE_SIZES)):
                rpp = tn // NPART
                x_ch = xinp.tile(
                    [NPART, rpp, WDIM], F32, name=f"xch{u}", tag=f"xch{u}"
                )
                nc.sync.dma_start(
                    x_ch[:],
                    x_d[n0 : n0 + tn, :].rearrange("(q c) w -> q c w", c=rpp),
                )
                x_chunks.append(x_ch)

            # Per-engine chaining of the elementwise ops in program order so
            # the scheduler finishes tile u before starting tile u+1 ops —
            # otherwise cross-tile interleaving delays the first out DMA.
            prev_op = {}

            def chain(key, bi):
                if key in prev_op:
                    add_dep_helper(
                        bi.ins, prev_op[key].ins, sync=False, reason="tile op order"
                    )
                prev_op[key] = bi

            for n0, tn in [
                (o, t) for _ in range(repeat) for o, t in zip(tile_offsets, TILE_SIZES)
            ]:
                u = tile_offsets.index(n0)
                rpp = tn // NPART
                half = tn // 2
                out_r = o_d[:, n0 : n0 + tn, :].rearrange(
                    "f (h j) p -> h f j p", h=2, j=half
                )  # (2, 64, half, P)

                # --- transpose row-groups: xT[w, c, j] = x[n0 + j*rpp + c, w]
                xt_ps = psump.tile([64, rpp, NPART], F32, name="xtp", tag="xtp")
                for c in range(rpp):
                    chain(
                        "pe",
                        nc.tensor.transpose(
                            xt_ps[:, c, :], x_chunks[u][:, c, :], ident[:]
                        ),
                    )
                # split transpose columns j = 64h + j' into partition halves
                xt_sb = workp.tile([NPART, rpp, 64], F32, tag="xt_sb")
                chain("v", nc.vector.tensor_copy(xt_sb[0:64], xt_ps[:, :, 0:64]))
                chain("v", nc.vector.tensor_copy(xt_sb[64:128], xt_ps[:, :, 64:128]))

                # --- s broadcast: one matmul, K=128; rhs streams (j', c) so
                # the free axis is n-order: m = j'*rpp + c ---
                s_ps = psump3.tile([NPART, half], F32, tag="s_ps")
                chain(
                    "pe",
                    nc.tensor.matmul(
                        s_ps[:], e_mat[:], xt_sb[:].rearrange("k c j -> k j c")
                    ),
                )

                # --- 25 per-p elementwise ops ---
                # out tiles are fp16: halves SBUF traffic, and the SWDGE
                # cast-DMA below writes f32 to HBM ~4x faster than the
                # f32 HWDGE path on this hardware (per-core DMA slice).
                odt = out_dt if out_dt is not None else (F32 if out_f32 else F16)
                out_t = outp.tile([NPART, half, P], odt, tag="out_t")
                if split == "act":
                    engmap = {p: "s" for p in range(P)}
                elif split == "dve":
                    engmap = {p: "v" for p in range(P)}
                elif split == "actdve":
                    engmap = {p: ("s" if p < 13 else "v") for p in range(P)}
                else:  # mixed
                    scalar_ps, gpsimd_ps = engine_split(tn)
                    engmap = {
                        p: ("s" if p in scalar_ps else "g" if p in gpsimd_ps else "v")
                        for p in range(P)
                    }
                need_ssb = any(e != "s" for e in engmap.values())
                if need_ssb:
                    s_sb = workp.tile([NPART, half], F32, tag="s_sb")
                    chain("v", nc.vector.tensor_copy(s_sb[:], s_ps[:]))
                for p in range(P):
                    e = engmap[p]
                    if e == "s":
                        bi = nc.scalar.activation(
                            out_t[:, :, p],
                            s_ps[:],
                            mybir.ActivationFunctionType.Relu,
                            bias=bias_arg,
                            scale=wcols[:, p : p + 1],
                        )
                    else:
                        eng = nc.gpsimd if e == "g" else nc.vector
                        bi = eng.tensor_scalar(
                            out_t[:, :, p],
                            s_sb[:],
                            wcols[:, p : p + 1],
                            0.0,
                            mybir.AluOpType.mult,
                            mybir.AluOpType.max,
                        )
                    chain(e, bi)

                if not skip_out_dma:
                    nc.gpsimd.dma_start(out_r, out_t[:])

    nc.compile()
    return nc


def build_bass_v2(
    with_bias: bool,
    repeat: int = 1,
    skip_out_dma: bool = False,
    tn: int = 512,
    sub: int = 64,
    evict: str = "act",  # "act" | "dve" | "split" | "none"
    out_dt=None,  # default F16
) -> bass.Bass:
    """v2: low-SBUF-traffic pipeline bound by the SWDGE cast output DMA.

    Per 512-row n-tile: PE transposes x and reduces over w via the block-
    diagonal ones matmul (s in PSUM, exact n order) -> DVE scalar_tensor_
    tensor multiplies a 25x-broadcast of s by the fp16 W pattern into PSUM
    (no SBUF write) -> ScalarE evicts PSUM->SBUF with fused relu + fp16
    cast (contiguous writes) -> one SWDGE dma_start casts fp16->f32 into
    HBM.  SBUF port traffic per tile is just: W-pattern read + fp16 out
    write + fp16 DMA read; the HWDGE rings carry only the small x loads.
    """
    nc = bacc.Bacc(None)

    x_d = nc.dram_tensor("x", (N_LOCAL, WDIM), F32, kind="ExternalInput")
    w_d = nc.dram_tensor("W", (F, P), F32, kind="ExternalInput")
    b_d = nc.dram_tensor("b", (F, 1), F32, kind="ExternalInput")
    o_d = nc.dram_tensor("out", (F, N_LOCAL, P), F32, kind="ExternalOutput")

    n_tiles = N_LOCAL // tn
    rpp = tn // NPART
    half = tn // 2
    n_sub = half // (sub // 2)   # sub-tiles per n-tile
    js = sub // 2                # j extent per sub-tile (per h half)
    odt = out_dt if out_dt is not None else F16

    with tile.TileContext(nc) as tc:
        with (
            tc.tile_pool(name="const", bufs=1) as constp,
            tc.tile_pool(name="xin", bufs=1) as xinp,
            tc.tile_pool(name="work", bufs=2) as workp,
            tc.tile_pool(name="outp", bufs=3) as outp,
            tc.tile_pool(name="ps_xt", bufs=2, space="PSUM") as ps_xt,
            tc.tile_pool(name="ps_s", bufs=2, space="PSUM") as ps_s,
            tc.tile_pool(name="ps_t", bufs=2, space="PSUM") as ps_t,
        ):
            ident = constp.tile([NPART, NPART], F32)
            masks.make_identity(nc, ident[:])

            e_mat = constp.tile([NPART, NPART], F32)
            nc.gpsimd.memset(e_mat[:], 0.0)
            nc.gpsimd.memset(e_mat[0:64, 0:64], 1.0)
            nc.gpsimd.memset(e_mat[64:128, 64:128], 1.0)

            # ACT table warm-up so the Relu table load is off the hot path
            warm = constp.tile([NPART, 1], F32)
            nc.vector.memset(warm[:], 0.0)
            warm_out = constp.tile([NPART, 1], F32)
            nc.scalar.activation(
                warm_out[:], warm[:], mybir.ActivationFunctionType.Relu
            )

            # fp16 W columns on both partition halves (cast on load), then
            # j-replicated W pattern for the stt multiplier
            wc16 = constp.tile([NPART, P], F16)
            nc.gpsimd.dma_start(wc16[0:64, :], w_d[:, :])
            nc.gpsimd.dma_start(wc16[64:128, :], w_d[:, :])
            wbig = constp.tile([NPART, js, P], F16)
            nc.vector.tensor_copy(
                wbig[:],
                wc16[:].rearrange("q (o p) -> q o p", o=1).to_broadcast(
                    (NPART, js, P)
                ),
            )
            if with_bias:
                bcol = constp.tile([NPART, 1], F32)
                nc.scalar.dma_start(bcol[0:64, :], b_d[:, :])
                nc.scalar.dma_start(bcol[64:128, :], b_d[:, :])
                bias_arg = bcol[:, 0:1]
            else:
                nc.gpsimd.dma_start(
                    constp.tile([1, 1], F32, name="bjunk")[:], b_d[0:1, :]
                )
                bias_arg = 0.0

            # --- load all x chunks up front (sync HWDGE ring, unused else) ---
            x_chunks = []
            for u in range(n_tiles):
                n0 = u * tn
                x_ch = xinp.tile(
                    [NPART, rpp, WDIM], F32, name=f"xch{u}", tag=f"xch{u}"
                )
                nc.sync.dma_start(
                    x_ch[:],
                    x_d[n0 : n0 + tn, :].rearrange("(q c) w -> q c w", c=rpp),
                )
                x_chunks.append(x_ch)

            prev_op = {}

            def chain(key, bi):
                if key in prev_op:
                    add_dep_helper(
                        bi.ins, prev_op[key].ins, sync=False, reason="tile op order"
                    )
                prev_op[key] = bi

            for u in [t for _ in range(repeat) for t in range(n_tiles)]:
                n0 = u * tn
                out_r = o_d[:, n0 : n0 + tn, :].rearrange(
                    "f (h j) p -> h f j p", h=2, j=half
                )

                # transpose row-groups: xt[w, c, j] = x[n0 + j*rpp + c, w]
                xt_ps = ps_xt.tile([64, rpp, NPART], F32, tag="xtp")
                for c in range(rpp):
                    chain(
                        "pe",
                        nc.tensor.transpose(
                            xt_ps[:, c, :], x_chunks[u][:, c, :], ident[:]
                        ),
                    )
                xt_sb = workp.tile([NPART, rpp, 64], F32, tag="xt_sb")
                chain("v", nc.vector.tensor_copy(xt_sb[0:64], xt_ps[:, :, 0:64]))
                chain("v", nc.vector.tensor_copy(xt_sb[64:128], xt_ps[:, :, 64:128]))

                # s broadcast over (h, f): s_ps[(h,f), m] = s[n0 + (tn/2)h + m]
                s_ps = ps_s.tile([NPART, half], F32, tag="s_ps")
                chain(
                    "pe",
                    nc.tensor.matmul(
                        s_ps[:], e_mat[:], xt_sb[:].rearrange("k c j -> k j c")
                    ),
                )

                out_t = outp.tile([NPART, half, P], odt, tag="out_t")
                for s in range(n_sub):
                    j0 = s * js
                    if evict == "none":
                        chain(
                            "v",
                            nc.vector.scalar_tensor_tensor(
                                out_t[:, j0 : j0 + js, :],
                                s_ps[:, j0 : j0 + js].to_broadcast((NPART, js, P)),
                                1.0,
                                wbig[:],
                                mybir.AluOpType.bypass,
                                mybir.AluOpType.mult,
                            ),
                        )
                        continue
                    t_ps = ps_t.tile([NPART, js, P], F32, tag="t_ps")
                    chain(
                        "v",
                        nc.vector.scalar_tensor_tensor(
                            t_ps[:],
                            s_ps[:, j0 : j0 + js].to_broadcast((NPART, js, P)),
                            1.0,
                            wbig[:],
                            mybir.AluOpType.bypass,
                            mybir.AluOpType.mult,
                        ),
                    )
                    use_act = (
                        evict == "act"
                        or (evict == "split" and s % 2 == 0)
                        or (evict == "split4" and s % 4 != 3)
                        or (evict == "split3" and s % 3 != 2)
                        or evict == "acts"
                    )
                    if use_act:
                        if evict == "acts":
                            bi = nc.scalar.tensor_scalar(
                                out_t[:, j0 : j0 + js, :],
                                t_ps[:],
                                0.0,
                                None,
                                mybir.AluOpType.max,
                            )
                        else:
                            bi = nc.scalar.activation(
                                out_t[:, j0 : j0 + js, :],
                                t_ps[:],
                                mybir.ActivationFunctionType.Relu,
                                bias=bias_arg,
                            )
                        chain("s", bi)
                    else:
                        chain(
                            "v",
                            nc.vector.tensor_scalar(
                                out_t[:, j0 : j0 + js, :],
                                t_ps[:],
                                0.0,
                                None,
                                mybir.AluOpType.max,
                            ),
                        )

                if not skip_out_dma:
                    nc.gpsimd.dma_start(out_r, out_t[:])

    nc.compile()
    return nc


def build_bass_v3(
    with_bias: bool,
    repeat: int = 1,
    skip_out_dma: bool = False,
    tile_sizes: tuple[int, ...] = (256, 256, 512, 512, 512, 512, 512, 512, 512),
    wbig_bcast: bool = True,
    relu_split: int = 2,   # subs per tile for the stt/relu/DMA sub-pipeline
    out_dt=None,
) -> bass.Bass:
    """v3: minimal-op pipeline — per n-tile ONE DVE stt produces the full
    (128, tn/2, 25) fp16 product tile in SBUF (in0 = s straight from PSUM
    broadcast over p, in1 = W broadcast over j), ScalarE applies relu in
    place, one SWDGE cast-DMA writes f32 HBM.  The PSUM round-trip for the
    product and the per-sub DVE/ACT ping-pong of v2 are gone, so the DVE
    chain (~7.6 us/512-tile) stays under the 9.1 us DMA transfer and the
    SWDGE queue never starves."""
    nc = bacc.Bacc(None)

    x_d = nc.dram_tensor("x", (N_LOCAL, WDIM), F32, kind="ExternalInput")
    w_d = nc.dram_tensor("W", (F, P), F32, kind="ExternalInput")
    b_d = nc.dram_tensor("b", (F, 1), F32, kind="ExternalInput")
    o_d = nc.dram_tensor("out", (F, N_LOCAL, P), F32, kind="ExternalOutput")

    assert sum(tile_sizes) == N_LOCAL
    odt = out_dt if out_dt is not None else F16
    max_half = max(tile_sizes) // 2

    with tile.TileContext(nc) as tc:
        with (
            tc.tile_pool(name="const", bufs=1) as constp,
            tc.tile_pool(name="xin", bufs=1) as xinp,
            tc.tile_pool(name="work", bufs=2) as workp,
            tc.tile_pool(name="outp", bufs=3) as outp,
            tc.tile_pool(name="ps_xt", bufs=2, space="PSUM") as ps_xt,
            tc.tile_pool(name="ps_s", bufs=2, space="PSUM") as ps_s,
        ):
            ident = constp.tile([NPART, NPART], F32)
            masks.make_identity(nc, ident[:])

            e_mat = constp.tile([NPART, NPART], F32)
            nc.gpsimd.memset(e_mat[:], 0.0)
            nc.gpsimd.memset(e_mat[0:64, 0:64], 1.0)
            nc.gpsimd.memset(e_mat[64:128, 64:128], 1.0)

            # fp16 W columns on both partition halves (cast on load)
            wc16 = constp.tile([NPART, P], F16)
            nc.gpsimd.dma_start(wc16[0:64, :], w_d[:, :])
            nc.gpsimd.dma_start(wc16[64:128, :], w_d[:, :])
            if wbig_bcast:
                w_in1 = wc16[:].rearrange("q (j p) -> q j p", j=1)
            else:
                wbig = constp.tile([NPART, max_half, P], F16)
                nc.vector.tensor_copy(
                    wbig[:],
                    wc16[:].rearrange("q (o p) -> q o p", o=1).to_broadcast(
                        (NPART, max_half, P)
                    ),
                )
            # ACT table warm-up: the per-tile relu runs on ScalarE
            warm = constp.tile([NPART, 1], F32)
            nc.vector.memset(warm[:], 0.0)
            warm_out = constp.tile([NPART, 1], F32)
            nc.scalar.activation(
                warm_out[:], warm[:], mybir.ActivationFunctionType.Relu
            )
            if with_bias:
                bcol = constp.tile([NPART, 1], F32)
                nc.scalar.dma_start(bcol[0:64, :], b_d[:, :])
                nc.scalar.dma_start(bcol[64:128, :], b_d[:, :])
                bias_arg = bcol[:, 0:1]
            else:
                nc.gpsimd.dma_start(
                    constp.tile([1, 1], F32, name="bjunk")[:], b_d[0:1, :]
                )
                bias_arg = 0.0

            tile_offsets = [sum(tile_sizes[:u]) for u in range(len(tile_sizes))]

            # --- load all x chunks up front (SP HWDGE ring, unused else) ---
            x_chunks = []
            for u, (n0, tn) in enumerate(zip(tile_offsets, tile_sizes)):
                rpp = tn // NPART
                x_ch = xinp.tile(
                    [NPART, rpp, WDIM], F32, name=f"xch{u}", tag=f"xch{u}"
                )
                nc.sync.dma_start(
                    x_ch[:],
                    x_d[n0 : n0 + tn, :].rearrange("(q c) w -> q c w", c=rpp),
                )
                x_chunks.append(x_ch)

            prev_op = {}

            def chain(key, bi):
                if key in prev_op:
                    add_dep_helper(
                        bi.ins, prev_op[key].ins, sync=False, reason="tile op order"
                    )
                prev_op[key] = bi

            # Software-pipelined: the s-pipeline (PE transposes -> ScalarE
            # column-split copies -> PE reduce matmul) for tile u+1 is
            # emitted BEFORE tile u's sub loop, so on every engine chain it
            # sits ahead of tile u's stt/relu work and the next tile's stt
            # can start the moment the previous tile's last stt retires.
            sched = [t for _ in range(repeat) for t in range(len(tile_sizes))]

            def s_stage(u):
                n0, tn = tile_offsets[u], tile_sizes[u]
                rpp = tn // NPART
                half = tn // 2
                xt_ps = ps_xt.tile([64, rpp, NPART], F32, tag="xtp")
                for c in range(rpp):
                    chain(
                        "pe",
                        nc.tensor.transpose(
                            xt_ps[:, c, :], x_chunks[u][:, c, :], ident[:]
                        ),
                    )
                xt_sb = workp.tile([NPART, rpp, 64], F32, tag="xt_sb")
                chain("s", nc.scalar.copy(xt_sb[0:64], xt_ps[:, :, 0:64]))
                chain("s", nc.scalar.copy(xt_sb[64:128], xt_ps[:, :, 64:128]))
                s_ps = ps_s.tile([NPART, half], F32, tag="s_ps")
                chain(
                    "pe",
                    nc.tensor.matmul(
                        s_ps[:], e_mat[:], xt_sb[:].rearrange("k c j -> k j c")
                    ),
                )
                return s_ps

            s_pipe = {0: s_stage(sched[0])}

            for w, u in enumerate(sched):
                n0, tn = tile_offsets[u], tile_sizes[u]
                half = tn // 2
                out_r = o_d[:, n0 : n0 + tn, :].rearrange(
                    "f (h j) p -> h f j p", h=2, j=half
                )
                if w + 1 < len(sched):
                    s_pipe[w + 1] = s_stage(sched[w + 1])
                s_ps = s_pipe.pop(w)

                # Per-sub pipeline: DVE product -> ScalarE relu (in place)
                # -> SWDGE cast-DMA.  Sub granularity lets the DMA of sub i
                # start while DVE works on sub i+1, hiding the relu latency.
                out_t = outp.tile([NPART, half, P], odt, tag="out_t")
                js = max(half // max(relu_split, 1), 64)
                n_sub = max(half // js, 1)
                js = half // n_sub
                for si in range(n_sub):
                    j0 = si * js
                    sl = out_t[:, j0 : j0 + js, :]
                    in1 = (
                        w_in1.to_broadcast((NPART, js, P))
                        if wbig_bcast
                        else wbig[:, 0:js, :]
                    )
                    chain(
                        "v",
                        nc.vector.scalar_tensor_tensor(
                            sl,
                            s_ps[:, j0 : j0 + js].to_broadcast((NPART, js, P)),
                            1.0,
                            in1,
                            mybir.AluOpType.bypass,
                            mybir.AluOpType.mult,
                        ),
                    )
                    chain(
                        "s",
                        nc.scalar.activation(
                            sl, sl, mybir.ActivationFunctionType.Relu, bias=bias_arg
                        ),
                    )
                    if not skip_out_dma:
                        nc.gpsimd.dma_start(
                            out_r[:, :, j0 : j0 + js, :],
                            out_t[:, j0 : j0 + js, :],
                        )

    nc.compile()
    return nc


_CACHE: dict[bool, bass.Bass] = {}


def _get_bass(with_bias: bool) -> bass.Bass:
    if with_bias not in _CACHE:
        _CACHE[with_bias] = build_bass_v3(
            with_bias,
            tile_sizes=(256, 256, 512, 512, 512, 512, 512, 512, 512),
            relu_split=4,
        )
    return _CACHE[with_bias]


last_exec_time_ns = None
last_profile = None


def kernel(x, W, b, trace=False, **run_kwargs):
    global last_exec_time_ns, last_profile
    x = np.ascontiguousarray(np.asarray(x, dtype=np.float32)).reshape(N_TOTAL, WDIM)
    wf = np.ascontiguousarray(np.asarray(W, dtype=np.float32)).reshape(F, P)
    bf = np.ascontiguousarray(np.asarray(b, dtype=np.float32)).reshape(F, 1)

    nc = _get_bass(bool(np.any(bf)))

    in_maps = [
        {
            "x": x[m * N_LOCAL : (m + 1) * N_LOCAL],
            "W": wf,
            "b": bf,
        }
        for m in range(N_CORES)
    ]
    res = run_bass_kernel_spmd(
        nc, in_maps, core_ids=list(range(N_CORES)), trace=trace, **run_kwargs
    )
    last_exec_time_ns = res.exec_time_ns
    last_profile = res.profile_json
    outs = [np.asarray(res.results[m]["out"]) for m in range(N_CORES)]
    full = np.concatenate(outs, axis=1)  # (F, N_TOTAL, P)
    return full[None]



# revision 12
# speedup vs baseline: 7.2074x; 7.2074x over previous
"""Trainium2 Bass kernel for nn_DeConvAfterDownSampling.

Math (from the reference): with s[n] = sum_w x[b,c,h,w] flattened over
n = (b,c,h), Wf = W.reshape(F, P):

    out[0, f, n, p] = relu(s[n] * Wf[f, p] + b[f])      # (1, F, N, P)

N = 8*64*64 = 32768, F = 64, P = 25.  Output is ~210 MB fp32 while inputs
are ~8 MB, so the kernel is bound by the output HBM write.

Sharding: data-parallel over n across 8 cores (N_LOCAL = 4096 per core);
W and b replicated; no cross-core communication.

Production kernel = build_bass_v3 (earlier iterations build_bass /
build_bass_v2 are kept for benchmarking comparisons).  Per-core plan
(partitions = (h, f) with h in {0,1} stacking two n-halves so all 128
partitions are used; n tiled as [256, 256, 512 x 7]):

  1. All x chunks DMA-load up front on the SP HWDGE ring (partition q
     holds rpp = tn/128 consecutive rows -> contiguous full-rate loads).
  2. s-pipeline, software-pipelined one tile ahead: PE transposes each
     (128 q, 64 w) row-group -> PSUM; ScalarE copies split the columns
     into partition halves; one K=128 PE matmul against a block-diagonal
     ones matrix reduces over w AND broadcasts s across all (h, f)
     partitions with the PSUM free axis in exact n order.
  3. Per 64-row sub-tile, ONE DVE scalar_tensor_tensor computes the full
     outer product s * W straight into fp16 SBUF (in0 = s from PSUM
     broadcast over p, in1 = the (128, 25) fp16 W tile broadcast over j
     with a stride-0 AP — nothing is materialized, no PSUM round-trip).
  4. ScalarE applies relu in place on the fp16 sub-tile (+b when b != 0).
  5. One SWDGE cast-DMA per sub-tile writes fp16 SBUF -> f32 HBM; the
     per-partition free layout (n-major, p-minor) is exactly contiguous
     HBM order.  SWDGE cast measures ~360 GB/s f32-side on this HW while
     the HWDGE f32 rings only do ~29 GB/s, so everything rides SWDGE.

Why this shape: the output write is the roofline (26.2 MB/core at
~360 GB/s = 72.8 us).  v2 ping-ponged DVE->PSUM->ScalarE per sub-tile,
which serialized into a 12 us/tile cadence and starved the DMA queue
(131 us/iter measured on HW).  v3 keeps the DVE chain at ~7.6 us/tile
(< 9.1 us DMA transfer per tile), so the SWDGE queue never starves:
~66-76 us/iter measured on HW via repeat-marginal timing, TimelineSim
span 85.3 us.  Rel err vs the jax reference: 5.2e-4 (fp16 W / fp16 out
path, gate is 2e-2).
"""

import numpy as np

import concourse.bass as bass
import concourse.mybir as mybir
from concourse import bacc, masks, tile
from concourse.bass_utils import run_bass_kernel_spmd
from concourse.tile_rust import add_dep_helper

F32 = mybir.dt.float32
F16 = mybir.dt.float16

N_CORES = 8
B, C, H, WDIM = 8, 64, 64, 64
F, P = 64, 25
N_TOTAL = B * C * H          # 32768
N_LOCAL = N_TOTAL // N_CORES  # 4096
# Per-tile n sizes: small first tiles shorten the pipeline-fill ramp;
# large tail tiles make the output DMAs 6.5 MB for real-HW DMA efficiency
# (descriptor overhead amortizes with transfer size).
TILE_SIZES = [256, 256, 512, 512, 512, 1024, 1024]
assert sum(TILE_SIZES) == N_LOCAL
NPART = 128

# Engine split for the 25 per-p elementwise ops (b == 0 fast path):
# ScalarE activation / GpSimd tensor_scalar / VectorE tensor_scalar.
# Small (ramp) tiles bias away from ScalarE, whose sequencer is slow.
def engine_split(tn):
    if tn <= 128:
        return set(range(3)), set(range(3, 10))
    if tn <= 256:
        return set(range(4)), set(range(4, 11))
    if tn == 512:
        return set(range(6)), set(range(6, 13))
    return set(range(8)), set(range(8, 14))


def build_bass(
    with_bias: bool,
    repeat: int = 1,
    skip_out_dma: bool = False,
    out_f32: bool = False,
    split: str = "act",  # "act" | "mixed" | "dve" | "actdve"
    out_dt=None,
) -> bass.Bass:
    """repeat > 1 re-executes the whole per-tile pipeline that many times
    (identical work, same output writes) — used only by the timing harness
    to make device time resolvable above the per-dispatch proxy overhead."""
    nc = bacc.Bacc(None)

    x_d = nc.dram_tensor("x", (N_LOCAL, WDIM), F32, kind="ExternalInput")
    w_d = nc.dram_tensor("W", (F, P), F32, kind="ExternalInput")
    b_d = nc.dram_tensor("b", (F, 1), F32, kind="ExternalInput")
    o_d = nc.dram_tensor("out", (F, N_LOCAL, P), F32, kind="ExternalOutput")

    with tile.TileContext(nc) as tc:
        with (
            tc.tile_pool(name="const", bufs=1) as constp,
            tc.tile_pool(name="xin", bufs=1) as xinp,
            tc.tile_pool(name="work", bufs=3) as workp,
            tc.tile_pool(name="outp", bufs=3) as outp,
            tc.tile_pool(name="psum", bufs=2, space="PSUM") as psump,
            tc.tile_pool(name="psum3", bufs=3, space="PSUM") as psump3,
        ):
            # Pool builds identity + E first: they gate the PE transposes.
            ident = constp.tile([NPART, NPART], F32)
            masks.make_identity(nc, ident[:])

            # Block-diagonal ones: E[k, i] = 1 iff k//64 == i//64.
            e_mat = constp.tile([NPART, NPART], F32)
            nc.gpsimd.memset(e_mat[:], 0.0)
            nc.gpsimd.memset(e_mat[0:64, 0:64], 1.0)
            nc.gpsimd.memset(e_mat[64:128, 64:128], 1.0)

            # Pull the ACT table load (~1.3 us) off the critical path: a
            # dummy Relu at t=0 makes insert_act_table_loads put it first.
            warm = constp.tile([NPART, 1], F32)
            nc.vector.memset(warm[:], 0.0)
            warm_out = constp.tile([NPART, 1], F32)
            nc.scalar.activation(
                warm_out[:], warm[:], mybir.ActivationFunctionType.Relu
            )

            # W columns replicated on both partition halves, via the ACT
            # HWDGE ring so neither the x loads (SP ring) nor the Pool
            # engine (identity/E) are delayed.
            wcols = constp.tile([NPART, P], F32)
            nc.scalar.dma_start(wcols[0:64, :], w_d[:, :])
            nc.scalar.dma_start(wcols[64:128, :], w_d[:, :])
            if with_bias:
                bcol = constp.tile([NPART, 1], F32)
                nc.scalar.dma_start(bcol[0:64, :], b_d[:, :])
                nc.scalar.dma_start(bcol[64:128, :], b_d[:, :])
                bias_arg = bcol[:, 0:1]
            else:
                # b is all zeros: skip the load, use an immediate bias.
                nc.gpsimd.dma_start(constp.tile([1, 1], F32, name="bjunk")[:],
                                    b_d[0:1, :])  # keep "b" a live input
                bias_arg = 0.0

            tile_offsets = [sum(TILE_SIZES[:u]) for u in range(len(TILE_SIZES))]

            # --- load x (critical path), one chunk per tile ---
            # Natural row order: partition q holds rpp = tn/128 consecutive
            # rows (rpp*256 B contiguous per partition -> full DMA rate).
            # n = n0 + q*rpp + c.
            x_chunks = []
            for u, (n0, tn) in enumerate(zip(tile_offsets, TILE_SIZES)):
                rpp = tn // NPART
                x_ch = xinp.tile(
                    [NPART, rpp, WDIM], F32, name=f"xch{u}", tag=f"xch{u}"
                )
                nc.sync.dma_start(
                    x_ch[:],
                    x_d[n0 : n0 + tn, :].rearrange("(q c) w -> q c w", c=rpp),
                )
                x_chunks.append(x_ch)

            # Per-engine chaining of the elementwise ops in program order so
            # the scheduler finishes tile u before starting tile u+1 ops —
            # otherwise cross-tile interleaving delays the first out DMA.
            prev_op = {}

            def chain(key, bi):
                if key in prev_op:
                    add_dep_helper(
                        bi.ins, prev_op[key].ins, sync=False, reason="tile op order"
                    )
                prev_op[key] = bi

            for n0, tn in [
                (o, t) for _ in range(repeat) for o, t in zip(tile_offsets, TILE_SIZES)
            ]:
                u = tile_offsets.index(n0)
                rpp = tn // NPART
                half = tn // 2
                out_r = o_d[:, n0 : n0 + tn, :].rearrange(
                    "f (h j) p -> h f j p", h=2, j=half
                )  # (2, 64, half, P)

                # --- transpose row-groups: xT[w, c, j] = x[n0 + j*rpp + c, w]
                xt_ps = psump.tile([64, rpp, NPART], F32, name="xtp", tag="xtp")
                for c in range(rpp):
                    chain(
                        "pe",
                        nc.tensor.transpose(
                            xt_ps[:, c, :], x_chunks[u][:, c, :], ident[:]
                        ),
                    )
                # split transpose columns j = 64h + j' into partition halves
                xt_sb = workp.tile([NPART, rpp, 64], F32, tag="xt_sb")
                chain("v", nc.vector.tensor_copy(xt_sb[0:64], xt_ps[:, :, 0:64]))
                chain("v", nc.vector.tensor_copy(xt_sb[64:128], xt_ps[:, :, 64:128]))

                # --- s broadcast: one matmul, K=128; rhs streams (j', c) so
                # the free axis is n-order: m = j'*rpp + c ---
                s_ps = psump3.tile([NPART, half], F32, tag="s_ps")
                chain(
                    "pe",
                    nc.tensor.matmul(
                        s_ps[:], e_mat[:], xt_sb[:].rearrange("k c j -> k j c")
                    ),
                )

                # --- 25 per-p elementwise ops ---
                # out tiles are fp16: halves SBUF traffic, and the SWDGE
                # cast-DMA below writes f32 to HBM ~4x faster than the
                # f32 HWDGE path on this hardware (per-core DMA slice).
                odt = out_dt if out_dt is not None else (F32 if out_f32 else F16)
                out_t = outp.tile([NPART, half, P], odt, tag="out_t")
                if split == "act":
                    engmap = {p: "s" for p in range(P)}
                elif split == "dve":
                    engmap = {p: "v" for p in range(P)}
                elif split == "actdve":
                    engmap = {p: ("s" if p < 13 else "v") for p in range(P)}
                else:  # mixed
                    scalar_ps, gpsimd_ps = engine_split(tn)
                    engmap = {
                        p: ("s" if p in scalar_ps else "g" if p in gpsimd_ps else "v")
                        for p in range(P)
                    }
                need_ssb = any(e != "s" for e in engmap.values())
                if need_ssb:
                    s_sb = workp.tile([NPART, half], F32, tag="s_sb")
                    chain("v", nc.vector.tensor_copy(s_sb[:], s_ps[:]))
                for p in range(P):
                    e = engmap[p]
                    if e == "s":
                        bi = nc.scalar.activation(
                            out_t[:, :, p],
                            s_ps[:],
                            mybir.ActivationFunctionType.Relu,
                            bias=bias_arg,
                            scale=wcols[:, p : p + 1],
                        )
                    else:
                        eng = nc.gpsimd if e == "g" else nc.vector
                        bi = eng.tensor_scalar(
                            out_t[:, :, p],
                            s_sb[:],
                            wcols[:, p : p + 1],
                            0.0,
                            mybir.AluOpType.mult,
                            mybir.AluOpType.max,
                        )
                    chain(e, bi)

                if not skip_out_dma:
                    nc.gpsimd.dma_start(out_r, out_t[:])

    nc.compile()
    return nc


def build_bass_v2(
    with_bias: bool,
    repeat: int = 1,
    skip_out_dma: bool = False,
    tn: int = 512,
    sub: int = 64,
    evict: str = "act",  # "act" | "dve" | "split" | "none"
    out_dt=None,  # default F16
) -> bass.Bass:
    """v2: low-SBUF-traffic pipeline bound by the SWDGE cast output DMA.

    Per 512-row n-tile: PE transposes x and reduces over w via the block-
    diagonal ones matmul (s in PSUM, exact n order) -> DVE scalar_tensor_
    tensor multiplies a 25x-broadcast of s by the fp16 W pattern into PSUM
    (no SBUF write) -> ScalarE evicts PSUM->SBUF with fused relu + fp16
    cast (contiguous writes) -> one SWDGE dma_start casts fp16->f32 into
    HBM.  SBUF port traffic per tile is just: W-pattern read + fp16 out
    write + fp16 DMA read; the HWDGE rings carry only the small x loads.
    """
    nc = bacc.Bacc(None)

    x_d = nc.dram_tensor("x", (N_LOCAL, WDIM), F32, kind="ExternalInput")
    w_d = nc.dram_tensor("W", (F, P), F32, kind="ExternalInput")
    b_d = nc.dram_tensor("b", (F, 1), F32, kind="ExternalInput")
    o_d = nc.dram_tensor("out", (F, N_LOCAL, P), F32, kind="ExternalOutput")

    n_tiles = N_LOCAL // tn
    rpp = tn // NPART
    half = tn // 2
    n_sub = half // (sub // 2)   # sub-tiles per n-tile
    js = sub // 2                # j extent per sub-tile (per h half)
    odt = out_dt if out_dt is not None else F16

    with tile.TileContext(nc) as tc:
        with (
            tc.tile_pool(name="const", bufs=1) as constp,
            tc.tile_pool(name="xin", bufs=1) as xinp,
            tc.tile_pool(name="work", bufs=2) as workp,
            tc.tile_pool(name="outp", bufs=3) as outp,
            tc.tile_pool(name="ps_xt", bufs=2, space="PSUM") as ps_xt,
            tc.tile_pool(name="ps_s", bufs=2, space="PSUM") as ps_s,
            tc.tile_pool(name="ps_t", bufs=2, space="PSUM") as ps_t,
        ):
            ident = constp.tile([NPART, NPART], F32)
            masks.make_identity(nc, ident[:])

            e_mat = constp.tile([NPART, NPART], F32)
            nc.gpsimd.memset(e_mat[:], 0.0)
            nc.gpsimd.memset(e_mat[0:64, 0:64], 1.0)
            nc.gpsimd.memset(e_mat[64:128, 64:128], 1.0)

            # ACT table warm-up so the Relu table load is off the hot path
            warm = constp.tile([NPART, 1], F32)
            nc.vector.memset(warm[:], 0.0)
            warm_out = constp.tile([NPART, 1], F32)
            nc.scalar.activation(
                warm_out[:], warm[:], mybir.ActivationFunctionType.Relu
            )

            # fp16 W columns on both partition halves (cast on load), then
            # j-replicated W pattern for the stt multiplier
            wc16 = constp.tile([NPART, P], F16)
            nc.gpsimd.dma_start(wc16[0:64, :], w_d[:, :])
            nc.gpsimd.dma_start(wc16[64:128, :], w_d[:, :])
            wbig = constp.tile([NPART, js, P], F16)
            nc.vector.tensor_copy(
                wbig[:],
                wc16[:].rearrange("q (o p) -> q o p", o=1).to_broadcast(
                    (NPART, js, P)
                ),
            )
            if with_bias:
                bcol = constp.tile([NPART, 1], F32)
                nc.scalar.dma_start(bcol[0:64, :], b_d[:, :])
                nc.scalar.dma_start(bcol[64:128, :], b_d[:, :])
                bias_arg = bcol[:, 0:1]
            else:
                nc.gpsimd.dma_start(
                    constp.tile([1, 1], F32, name="bjunk")[:], b_d[0:1, :]
                )
                bias_arg = 0.0

            # --- load all x chunks up front (sync HWDGE ring, unused else) ---
            x_chunks = []
            for u in range(n_tiles):
                n0 = u * tn
                x_ch = xinp.tile(
                    [NPART, rpp, WDIM], F32, name=f"xch{u}", tag=f"xch{u}"
                )
                nc.sync.dma_start(
                    x_ch[:],
                    x_d[n0 : n0 + tn, :].rearrange("(q c) w -> q c w", c=rpp),
                )
                x_chunks.append(x_ch)

            prev_op = {}

            def chain(key, bi):
                if key in prev_op:
                    add_dep_helper(
                        bi.ins, prev_op[key].ins, sync=False, reason="tile op order"
                    )
                prev_op[key] = bi

            for u in [t for _ in range(repeat) for t in range(n_tiles)]:
                n0 = u * tn
                out_r = o_d[:, n0 : n0 + tn, :].rearrange(
                    "f (h j) p -> h f j p", h=2, j=half
                )

                # transpose row-groups: xt[w, c, j] = x[n0 + j*rpp + c, w]
                xt_ps = ps_xt.tile([64, rpp, NPART], F32, tag="xtp")
                for c in range(rpp):
                    chain(
                        "pe",
                        nc.tensor.transpose(
                            xt_ps[:, c, :], x_chunks[u][:, c, :], ident[:]
                        ),
                    )
                xt_sb = workp.tile([NPART, rpp, 64], F32, tag="xt_sb")
                chain("v", nc.vector.tensor_copy(xt_sb[0:64], xt_ps[:, :, 0:64]))
                chain("v", nc.vector.tensor_copy(xt_sb[64:128], xt_ps[:, :, 64:128]))

                # s broadcast over (h, f): s_ps[(h,f), m] = s[n0 + (tn/2)h + m]
                s_ps = ps_s.tile([NPART, half], F32, tag="s_ps")
                chain(
                    "pe",
                    nc.tensor.matmul(
                        s_ps[:], e_mat[:], xt_sb[:].rearrange("k c j -> k j c")
                    ),
                )

                out_t = outp.tile([NPART, half, P], odt, tag="out_t")
                for s in range(n_sub):
                    j0 = s * js
                    if evict == "none":
                        chain(
                            "v",
                            nc.vector.scalar_tensor_tensor(
                                out_t[:, j0 : j0 + js, :],
                                s_ps[:, j0 : j0 + js].to_broadcast((NPART, js, P)),
                                1.0,
                                wbig[:],
                                mybir.AluOpType.bypass,
                                mybir.AluOpType.mult,
                            ),
                        )
                        continue
                    t_ps = ps_t.tile([NPART, js, P], F32, tag="t_ps")
                    chain(
                        "v",
                        nc.vector.scalar_tensor_tensor(
                            t_ps[:],
                            s_ps[:, j0 : j0 + js].to_broadcast((NPART, js, P)),
                            1.0,
                            wbig[:],
                            mybir.AluOpType.bypass,
                            mybir.AluOpType.mult,
                        ),
                    )
                    use_act = (
                        evict == "act"
                        or (evict == "split" and s % 2 == 0)
                        or (evict == "split4" and s % 4 != 3)
                        or (evict == "split3" and s % 3 != 2)
                        or evict == "acts"
                    )
                    if use_act:
                        if evict == "acts":
                            bi = nc.scalar.tensor_scalar(
                                out_t[:, j0 : j0 + js, :],
                                t_ps[:],
                                0.0,
                                None,
                                mybir.AluOpType.max,
                            )
                        else:
                            bi = nc.scalar.activation(
                                out_t[:, j0 : j0 + js, :],
                                t_ps[:],
                                mybir.ActivationFunctionType.Relu,
                                bias=bias_arg,
                            )
                        chain("s", bi)
                    else:
                        chain(
                            "v",
                            nc.vector.tensor_scalar(
                                out_t[:, j0 : j0 + js, :],
                                t_ps[:],
                                0.0,
                                None,
                                mybir.AluOpType.max,
                            ),
                        )

                if not skip_out_dma:
                    nc.gpsimd.dma_start(out_r, out_t[:])

    nc.compile()
    return nc


def build_bass_v3(
    with_bias: bool,
    repeat: int = 1,
    skip_out_dma: bool = False,
    tile_sizes: tuple[int, ...] = (256, 256, 512, 512, 512, 512, 512, 512, 512),
    wbig_bcast: bool = True,
    relu_split: int = 4,   # stt/relu subs per tile (production granularity)
    dma_split: int = 4,    # output DMAs per tile (measured: fine > coarse in-kernel)
    out_dt=None,
) -> bass.Bass:
    """v3: minimal-op pipeline — per n-tile ONE DVE stt produces the full
    (128, tn/2, 25) fp16 product tile in SBUF (in0 = s straight from PSUM
    broadcast over p, in1 = W broadcast over j), ScalarE applies relu in
    place, one SWDGE cast-DMA writes f32 HBM.  The PSUM round-trip for the
    product and the per-sub DVE/ACT ping-pong of v2 are gone, so the DVE
    chain (~7.6 us/512-tile) stays under the 9.1 us DMA transfer and the
    SWDGE queue never starves."""
    nc = bacc.Bacc(None)

    x_d = nc.dram_tensor("x", (N_LOCAL, WDIM), F32, kind="ExternalInput")
    w_d = nc.dram_tensor("W", (F, P), F32, kind="ExternalInput")
    b_d = nc.dram_tensor("b", (F, 1), F32, kind="ExternalInput")
    o_d = nc.dram_tensor("out", (F, N_LOCAL, P), F32, kind="ExternalOutput")

    assert sum(tile_sizes) == N_LOCAL
    odt = out_dt if out_dt is not None else F16
    max_half = max(tile_sizes) // 2

    with tile.TileContext(nc) as tc:
        with (
            tc.tile_pool(name="const", bufs=1) as constp,
            tc.tile_pool(name="xin", bufs=1) as xinp,
            tc.tile_pool(name="work", bufs=2) as workp,
            tc.tile_pool(name="outp", bufs=3) as outp,
            tc.tile_pool(name="ps_xt", bufs=2, space="PSUM") as ps_xt,
            tc.tile_pool(name="ps_s", bufs=2, space="PSUM") as ps_s,
        ):
            ident = constp.tile([NPART, NPART], F32)
            masks.make_identity(nc, ident[:])

            e_mat = constp.tile([NPART, NPART], F32)
            nc.gpsimd.memset(e_mat[:], 0.0)
            nc.gpsimd.memset(e_mat[0:64, 0:64], 1.0)
            nc.gpsimd.memset(e_mat[64:128, 64:128], 1.0)

            # fp16 W columns on both partition halves (cast on load)
            wc16 = constp.tile([NPART, P], F16)
            nc.gpsimd.dma_start(wc16[0:64, :], w_d[:, :])
            nc.gpsimd.dma_start(wc16[64:128, :], w_d[:, :])
            if wbig_bcast:
                w_in1 = wc16[:].rearrange("q (j p) -> q j p", j=1)
            else:
                wbig = constp.tile([NPART, max_half, P], F16)
                nc.vector.tensor_copy(
                    wbig[:],
                    wc16[:].rearrange("q (o p) -> q o p", o=1).to_broadcast(
                        (NPART, max_half, P)
                    ),
                )
            # ACT table warm-up: the per-tile relu runs on ScalarE
            warm = constp.tile([NPART, 1], F32)
            nc.vector.memset(warm[:], 0.0)
            warm_out = constp.tile([NPART, 1], F32)
            nc.scalar.activation(
                warm_out[:], warm[:], mybir.ActivationFunctionType.Relu
            )
            if with_bias:
                bcol = constp.tile([NPART, 1], F32)
                nc.scalar.dma_start(bcol[0:64, :], b_d[:, :])
                nc.scalar.dma_start(bcol[64:128, :], b_d[:, :])
                bias_arg = bcol[:, 0:1]
            else:
                nc.gpsimd.dma_start(
                    constp.tile([1, 1], F32, name="bjunk")[:], b_d[0:1, :]
                )
                bias_arg = 0.0

            tile_offsets = [sum(tile_sizes[:u]) for u in range(len(tile_sizes))]

            # --- load all x chunks up front (SP HWDGE ring, unused else) ---
            x_chunks = []
            for u, (n0, tn) in enumerate(zip(tile_offsets, tile_sizes)):
                rpp = tn // NPART
                x_ch = xinp.tile(
                    [NPART, rpp, WDIM], F32, name=f"xch{u}", tag=f"xch{u}"
                )
                nc.sync.dma_start(
                    x_ch[:],
                    x_d[n0 : n0 + tn, :].rearrange("(q c) w -> q c w", c=rpp),
                )
                x_chunks.append(x_ch)

            prev_op = {}

            def chain(key, bi):
                if key in prev_op:
                    add_dep_helper(
                        bi.ins, prev_op[key].ins, sync=False, reason="tile op order"
                    )
                prev_op[key] = bi

            # Software-pipelined: the s-pipeline (PE transposes -> ScalarE
            # column-split copies -> PE reduce matmul) for tile u+1 is
            # emitted BEFORE tile u's sub loop, so on every engine chain it
            # sits ahead of tile u's stt/relu work and the next tile's stt
            # can start the moment the previous tile's last stt retires.
            sched = [t for _ in range(repeat) for t in range(len(tile_sizes))]

            def s_stage(u):
                n0, tn = tile_offsets[u], tile_sizes[u]
                rpp = tn // NPART
                half = tn // 2
                xt_ps = ps_xt.tile([64, rpp, NPART], F32, tag="xtp")
                for c in range(rpp):
                    chain(
                        "pe",
                        nc.tensor.transpose(
                            xt_ps[:, c, :], x_chunks[u][:, c, :], ident[:]
                        ),
                    )
                xt_sb = workp.tile([NPART, rpp, 64], F32, tag="xt_sb")
                chain("s", nc.scalar.copy(xt_sb[0:64], xt_ps[:, :, 0:64]))
                chain("s", nc.scalar.copy(xt_sb[64:128], xt_ps[:, :, 64:128]))
                s_ps = ps_s.tile([NPART, half], F32, tag="s_ps")
                chain(
                    "pe",
                    nc.tensor.matmul(
                        s_ps[:], e_mat[:], xt_sb[:].rearrange("k c j -> k j c")
                    ),
                )
                return s_ps

            s_pipe = {0: s_stage(sched[0])}

            for w, u in enumerate(sched):
                n0, tn = tile_offsets[u], tile_sizes[u]
                half = tn // 2
                out_r = o_d[:, n0 : n0 + tn, :].rearrange(
                    "f (h j) p -> h f j p", h=2, j=half
                )
                if w + 1 < len(sched):
                    s_pipe[w + 1] = s_stage(sched[w + 1])
                s_ps = s_pipe.pop(w)

                # Per-sub production: DVE product -> ScalarE relu (in
                # place), fine-grained so the DVE/ACT chains pipeline; DMA
                # granularity is independent (dma_split groups per tile —
                # bigger DMAs measure faster on this HW: 8x3.3MB SWDGE runs
                # ~57us/26MB vs ~73us for 36x0.8MB).
                out_t = outp.tile([NPART, half, P], odt, tag="out_t")
                js = max(half // max(relu_split, 1), 64)
                n_sub = max(half // js, 1)
                js = half // n_sub
                n_dma = min(max(dma_split, 1), n_sub)
                subs_per_dma = (n_sub + n_dma - 1) // n_dma
                for si in range(n_sub):
                    j0 = si * js
                    sl = out_t[:, j0 : j0 + js, :]
                    in1 = (
                        w_in1.to_broadcast((NPART, js, P))
                        if wbig_bcast
                        else wbig[:, 0:js, :]
                    )
                    chain(
                        "v",
                        nc.vector.scalar_tensor_tensor(
                            sl,
                            s_ps[:, j0 : j0 + js].to_broadcast((NPART, js, P)),
                            1.0,
                            in1,
                            mybir.AluOpType.bypass,
                            mybir.AluOpType.mult,
                        ),
                    )
                    chain(
                        "s",
                        nc.scalar.activation(
                            sl, sl, mybir.ActivationFunctionType.Relu, bias=bias_arg
                        ),
                    )
                    if not skip_out_dma and (si + 1) % subs_per_dma == 0:
                        g0 = (si + 1 - subs_per_dma) * js
                        g1 = (si + 1) * js
                        nc.gpsimd.dma_start(
                            out_r[:, :, g0:g1, :], out_t[:, g0:g1, :]
                        )

    nc.compile()
    return nc


_CACHE: dict[bool, bass.Bass] = {}


# Production configuration (single source of truth for _get_bass and the
# timing harness's repeat builds).
PROD_CFG = dict(
    tile_sizes=(256, 256, 512, 512, 512, 512, 512, 512, 512),
    relu_split=4,
    dma_split=4,
)


def _build_timing(with_bias: bool, repeat: int = 1) -> bass.Bass:
    """Production kernel with the per-tile pipeline repeated `repeat` times
    (identical work each repeat) — lets the timing harness cancel the
    per-dispatch proxy overhead via a repeat-marginal measurement."""
    return build_bass_v3(with_bias, repeat=repeat, **PROD_CFG)


def _get_bass(with_bias: bool) -> bass.Bass:
    if with_bias not in _CACHE:
        _CACHE[with_bias] = build_bass_v3(with_bias, **PROD_CFG)
    return _CACHE[with_bias]


last_exec_time_ns = None
last_profile = None


def kernel(x, W, b, trace=False, **run_kwargs):
    global last_exec_time_ns, last_profile
    x = np.ascontiguousarray(np.asarray(x, dtype=np.float32)).reshape(N_TOTAL, WDIM)
    wf = np.ascontiguousarray(np.asarray(W, dtype=np.float32)).reshape(F, P)
    bf = np.ascontiguousarray(np.asarray(b, dtype=np.float32)).reshape(F, 1)

    nc = _get_bass(bool(np.any(bf)))

    in_maps = [
        {
            "x": x[m * N_LOCAL : (m + 1) * N_LOCAL],
            "W": wf,
            "b": bf,
        }
        for m in range(N_CORES)
    ]
    res = run_bass_kernel_spmd(
        nc, in_maps, core_ids=list(range(N_CORES)), trace=trace, **run_kwargs
    )
    last_exec_time_ns = res.exec_time_ns
    last_profile = res.profile_json
    outs = [np.asarray(res.results[m]["out"]) for m in range(N_CORES)]
    full = np.concatenate(outs, axis=1)  # (F, N_TOTAL, P)
    return full[None]



# revision 16
# speedup vs baseline: 10.6039x; 1.4713x over previous
"""Trainium2 Bass kernel for nn_DeConvAfterDownSampling.

Math (from the reference): with s[n] = sum_w x[b,c,h,w] flattened over
n = (b,c,h), Wf = W.reshape(F, P):

    out[0, f, n, p] = relu(s[n] * Wf[f, p] + b[f])      # (1, F, N, P)

N = 8*64*64 = 32768, F = 64, P = 25.  Output is ~210 MB fp32 while inputs
are ~8 MB, so the kernel is bound by the output HBM write.

Sharding: data-parallel over n across 8 cores (N_LOCAL = 4096 per core);
W and b replicated; no cross-core communication.

Production kernel = build_bass_v3 (earlier iterations build_bass /
build_bass_v2 are kept for benchmarking comparisons).  Per-core plan
(partitions = (h, f) with h in {0,1} stacking two n-halves so all 128
partitions are used; n tiled as [256, 256, 512 x 7]):

  1. All x chunks DMA-load up front on the SP HWDGE ring (partition q
     holds rpp = tn/128 consecutive rows -> contiguous full-rate loads).
  2. s-pipeline, software-pipelined one tile ahead: PE transposes each
     (128 q, 64 w) row-group -> PSUM; ScalarE copies split the columns
     into partition halves; one K=128 PE matmul against a block-diagonal
     ones matrix reduces over w AND broadcasts s across all (h, f)
     partitions with the PSUM free axis in exact n order.
  3. Per 64-row sub-tile, ONE DVE scalar_tensor_tensor computes the full
     outer product s * W straight into fp16 SBUF (in0 = s from PSUM
     broadcast over p, in1 = the (128, 25) fp16 W tile broadcast over j
     with a stride-0 AP — nothing is materialized, no PSUM round-trip).
  4. ScalarE applies relu in place on the fp16 sub-tile (+b when b != 0).
  5. One SWDGE cast-DMA per sub-tile writes fp16 SBUF -> f32 HBM; the
     per-partition free layout (n-major, p-minor) is exactly contiguous
     HBM order.  SWDGE cast measures ~360 GB/s f32-side on this HW while
     the HWDGE f32 rings only do ~29 GB/s, so everything rides SWDGE.

Why this shape: the output write is the roofline (26.2 MB/core at
~360 GB/s = 72.8 us).  v2 ping-ponged DVE->PSUM->ScalarE per sub-tile,
which serialized into a 12 us/tile cadence and starved the DMA queue
(131 us/iter measured on HW).  v3 keeps the DVE chain at ~7.6 us/tile
(< 9.1 us DMA transfer per tile), so the SWDGE queue never starves:
~60-77 us/iter measured on HW via paired repeat-marginal timing (v2
baseline 131 us, pure-DMA floor ~53-73 us), TimelineSim span 85.3 us.
Rel err vs the jax reference: 5.2e-4 (fp16 W / fp16 out path, gate 2e-2).
"""

import numpy as np

import concourse.bass as bass
import concourse.mybir as mybir
from concourse import bacc, masks, tile
from concourse.bass_utils import run_bass_kernel_spmd
from concourse.tile_rust import add_dep_helper

F32 = mybir.dt.float32
F16 = mybir.dt.float16

N_CORES = 8
B, C, H, WDIM = 8, 64, 64, 64
F, P = 64, 25
N_TOTAL = B * C * H          # 32768
N_LOCAL = N_TOTAL // N_CORES  # 4096
# Per-tile n sizes: small first tiles shorten the pipeline-fill ramp;
# large tail tiles make the output DMAs 6.5 MB for real-HW DMA efficiency
# (descriptor overhead amortizes with transfer size).
TILE_SIZES = [256, 256, 512, 512, 512, 1024, 1024]
assert sum(TILE_SIZES) == N_LOCAL
NPART = 128

# Engine split for the 25 per-p elementwise ops (b == 0 fast path):
# ScalarE activation / GpSimd tensor_scalar / VectorE tensor_scalar.
# Small (ramp) tiles bias away from ScalarE, whose sequencer is slow.
def engine_split(tn):
    if tn <= 128:
        return set(range(3)), set(range(3, 10))
    if tn <= 256:
        return set(range(4)), set(range(4, 11))
    if tn == 512:
        return set(range(6)), set(range(6, 13))
    return set(range(8)), set(range(8, 14))


def build_bass(
    with_bias: bool,
    repeat: int = 1,
    skip_out_dma: bool = False,
    out_f32: bool = False,
    split: str = "act",  # "act" | "mixed" | "dve" | "actdve"
    out_dt=None,
) -> bass.Bass:
    """repeat > 1 re-executes the whole per-tile pipeline that many times
    (identical work, same output writes) — used only by the timing harness
    to make device time resolvable above the per-dispatch proxy overhead."""
    nc = bacc.Bacc(None)

    x_d = nc.dram_tensor("x", (N_LOCAL, WDIM), F32, kind="ExternalInput")
    w_d = nc.dram_tensor("W", (F, P), F32, kind="ExternalInput")
    b_d = nc.dram_tensor("b", (F, 1), F32, kind="ExternalInput")
    o_d = nc.dram_tensor("out", (F, N_LOCAL, P), F32, kind="ExternalOutput")

    with tile.TileContext(nc) as tc:
        with (
            tc.tile_pool(name="const", bufs=1) as constp,
            tc.tile_pool(name="xin", bufs=1) as xinp,
            tc.tile_pool(name="work", bufs=3) as workp,
            tc.tile_pool(name="outp", bufs=out_bufs) as outp,
            tc.tile_pool(name="prodp", bufs=2) as prodp,
            tc.tile_pool(name="psum", bufs=2, space="PSUM") as psump,
            tc.tile_pool(name="psum3", bufs=3, space="PSUM") as psump3,
        ):
            # Pool builds identity + E first: they gate the PE transposes.
            ident = constp.tile([NPART, NPART], F32)
            masks.make_identity(nc, ident[:])

            # Block-diagonal ones: E[k, i] = 1 iff k//64 == i//64.
            e_mat = constp.tile([NPART, NPART], F32)
            nc.gpsimd.memset(e_mat[:], 0.0)
            nc.gpsimd.memset(e_mat[0:64, 0:64], 1.0)
            nc.gpsimd.memset(e_mat[64:128, 64:128], 1.0)

            # Pull the ACT table load (~1.3 us) off the critical path: a
            # dummy Relu at t=0 makes insert_act_table_loads put it first.
            warm = constp.tile([NPART, 1], F32)
            nc.vector.memset(warm[:], 0.0)
            warm_out = constp.tile([NPART, 1], F32)
            nc.scalar.activation(
                warm_out[:], warm[:], mybir.ActivationFunctionType.Relu
            )

            # W columns replicated on both partition halves, via the ACT
            # HWDGE ring so neither the x loads (SP ring) nor the Pool
            # engine (identity/E) are delayed.
            wcols = constp.tile([NPART, P], F32)
            nc.scalar.dma_start(wcols[0:64, :], w_d[:, :])
            nc.scalar.dma_start(wcols[64:128, :], w_d[:, :])
            if with_bias:
                bcol = constp.tile([NPART, 1], F32)
                nc.scalar.dma_start(bcol[0:64, :], b_d[:, :])
                nc.scalar.dma_start(bcol[64:128, :], b_d[:, :])
                bias_arg = bcol[:, 0:1]
            else:
                # b is all zeros: skip the load, use an immediate bias.
                nc.gpsimd.dma_start(constp.tile([1, 1], F32, name="bjunk")[:],
                                    b_d[0:1, :])  # keep "b" a live input
                bias_arg = 0.0

            tile_offsets = [sum(TILE_SIZES[:u]) for u in range(len(TILE_SIZES))]

            # --- load x (critical path), one chunk per tile ---
            # Natural row order: partition q holds rpp = tn/128 consecutive
            # rows (rpp*256 B contiguous per partition -> full DMA rate).
            # n = n0 + q*rpp + c.
            x_chunks = []
            for u, (n0, tn) in enumerate(zip(tile_offsets, TILE_SIZES)):
                rpp = tn // NPART
                x_ch = xinp.tile(
                    [NPART, rpp, WDIM], F32, name=f"xch{u}", tag=f"xch{u}"
                )
                nc.sync.dma_start(
                    x_ch[:],
                    x_d[n0 : n0 + tn, :].rearrange("(q c) w -> q c w", c=rpp),
                )
                x_chunks.append(x_ch)

            # Per-engine chaining of the elementwise ops in program order so
            # the scheduler finishes tile u before starting tile u+1 ops —
            # otherwise cross-tile interleaving delays the first out DMA.
            prev_op = {}

            def chain(key, bi):
                if key in prev_op:
                    add_dep_helper(
                        bi.ins, prev_op[key].ins, sync=False, reason="tile op order"
                    )
                prev_op[key] = bi

            for n0, tn in [
                (o, t) for _ in range(repeat) for o, t in zip(tile_offsets, TILE_SIZES)
            ]:
                u = tile_offsets.index(n0)
                rpp = tn // NPART
                half = tn // 2
                out_r = o_d[:, n0 : n0 + tn, :].rearrange(
                    "f (h j) p -> h f j p", h=2, j=half
                )  # (2, 64, half, P)

                # --- transpose row-groups: xT[w, c, j] = x[n0 + j*rpp + c, w]
                xt_ps = psump.tile([64, rpp, NPART], F32, name="xtp", tag="xtp")
                for c in range(rpp):
                    chain(
                        "pe",
                        nc.tensor.transpose(
                            xt_ps[:, c, :], x_chunks[u][:, c, :], ident[:]
                        ),
                    )
                # split transpose columns j = 64h + j' into partition halves
                xt_sb = workp.tile([NPART, rpp, 64], F32, tag="xt_sb")
                chain("v", nc.vector.tensor_copy(xt_sb[0:64], xt_ps[:, :, 0:64]))
                chain("v", nc.vector.tensor_copy(xt_sb[64:128], xt_ps[:, :, 64:128]))

                # --- s broadcast: one matmul, K=128; rhs streams (j', c) so
                # the free axis is n-order: m = j'*rpp + c ---
                s_ps = psump3.tile([NPART, half], F32, tag="s_ps")
                chain(
                    "pe",
                    nc.tensor.matmul(
                        s_ps[:], e_mat[:], xt_sb[:].rearrange("k c j -> k j c")
                    ),
                )

                # --- 25 per-p elementwise ops ---
                # out tiles are fp16: halves SBUF traffic, and the SWDGE
                # cast-DMA below writes f32 to HBM ~4x faster than the
                # f32 HWDGE path on this hardware (per-core DMA slice).
                odt = out_dt if out_dt is not None else (F32 if out_f32 else F16)
                out_t = outp.tile([NPART, half, P], odt, tag="out_t")
                if split == "act":
                    engmap = {p: "s" for p in range(P)}
                elif split == "dve":
                    engmap = {p: "v" for p in range(P)}
                elif split == "actdve":
                    engmap = {p: ("s" if p < 13 else "v") for p in range(P)}
                else:  # mixed
                    scalar_ps, gpsimd_ps = engine_split(tn)
                    engmap = {
                        p: ("s" if p in scalar_ps else "g" if p in gpsimd_ps else "v")
                        for p in range(P)
                    }
                need_ssb = any(e != "s" for e in engmap.values())
                if need_ssb:
                    s_sb = workp.tile([NPART, half], F32, tag="s_sb")
                    chain("v", nc.vector.tensor_copy(s_sb[:], s_ps[:]))
                for p in range(P):
                    e = engmap[p]
                    if e == "s":
                        bi = nc.scalar.activation(
                            out_t[:, :, p],
                            s_ps[:],
                            mybir.ActivationFunctionType.Relu,
                            bias=bias_arg,
                            scale=wcols[:, p : p + 1],
                        )
                    else:
                        eng = nc.gpsimd if e == "g" else nc.vector
                        bi = eng.tensor_scalar(
                            out_t[:, :, p],
                            s_sb[:],
                            wcols[:, p : p + 1],
                            0.0,
                            mybir.AluOpType.mult,
                            mybir.AluOpType.max,
                        )
                    chain(e, bi)

                if not skip_out_dma:
                    nc.gpsimd.dma_start(out_r, out_t[:])

    nc.compile()
    return nc


def build_bass_v2(
    with_bias: bool,
    repeat: int = 1,
    skip_out_dma: bool = False,
    tn: int = 512,
    sub: int = 64,
    evict: str = "act",  # "act" | "dve" | "split" | "none"
    out_dt=None,  # default F16
) -> bass.Bass:
    """v2: low-SBUF-traffic pipeline bound by the SWDGE cast output DMA.

    Per 512-row n-tile: PE transposes x and reduces over w via the block-
    diagonal ones matmul (s in PSUM, exact n order) -> DVE scalar_tensor_
    tensor multiplies a 25x-broadcast of s by the fp16 W pattern into PSUM
    (no SBUF write) -> ScalarE evicts PSUM->SBUF with fused relu + fp16
    cast (contiguous writes) -> one SWDGE dma_start casts fp16->f32 into
    HBM.  SBUF port traffic per tile is just: W-pattern read + fp16 out
    write + fp16 DMA read; the HWDGE rings carry only the small x loads.
    """
    nc = bacc.Bacc(None)

    x_d = nc.dram_tensor("x", (N_LOCAL, WDIM), F32, kind="ExternalInput")
    w_d = nc.dram_tensor("W", (F, P), F32, kind="ExternalInput")
    b_d = nc.dram_tensor("b", (F, 1), F32, kind="ExternalInput")
    o_d = nc.dram_tensor("out", (F, N_LOCAL, P), F32, kind="ExternalOutput")

    n_tiles = N_LOCAL // tn
    rpp = tn // NPART
    half = tn // 2
    n_sub = half // (sub // 2)   # sub-tiles per n-tile
    js = sub // 2                # j extent per sub-tile (per h half)
    odt = out_dt if out_dt is not None else F16

    with tile.TileContext(nc) as tc:
        with (
            tc.tile_pool(name="const", bufs=1) as constp,
            tc.tile_pool(name="xin", bufs=1) as xinp,
            tc.tile_pool(name="work", bufs=2) as workp,
            tc.tile_pool(name="outp", bufs=out_bufs) as outp,
            tc.tile_pool(name="prodp", bufs=2) as prodp,
            tc.tile_pool(name="ps_xt", bufs=2, space="PSUM") as ps_xt,
            tc.tile_pool(name="ps_s", bufs=2, space="PSUM") as ps_s,
            tc.tile_pool(name="ps_t", bufs=2, space="PSUM") as ps_t,
        ):
            ident = constp.tile([NPART, NPART], F32)
            masks.make_identity(nc, ident[:])

            e_mat = constp.tile([NPART, NPART], F32)
            nc.gpsimd.memset(e_mat[:], 0.0)
            nc.gpsimd.memset(e_mat[0:64, 0:64], 1.0)
            nc.gpsimd.memset(e_mat[64:128, 64:128], 1.0)

            # ACT table warm-up so the Relu table load is off the hot path
            warm = constp.tile([NPART, 1], F32)
            nc.vector.memset(warm[:], 0.0)
            warm_out = constp.tile([NPART, 1], F32)
            nc.scalar.activation(
                warm_out[:], warm[:], mybir.ActivationFunctionType.Relu
            )

            # fp16 W columns on both partition halves (cast on load), then
            # j-replicated W pattern for the stt multiplier
            wc16 = constp.tile([NPART, P], F16)
            nc.gpsimd.dma_start(wc16[0:64, :], w_d[:, :])
            nc.gpsimd.dma_start(wc16[64:128, :], w_d[:, :])
            wbig = constp.tile([NPART, js, P], F16)
            nc.vector.tensor_copy(
                wbig[:],
                wc16[:].rearrange("q (o p) -> q o p", o=1).to_broadcast(
                    (NPART, js, P)
                ),
            )
            if with_bias:
                bcol = constp.tile([NPART, 1], F32)
                nc.scalar.dma_start(bcol[0:64, :], b_d[:, :])
                nc.scalar.dma_start(bcol[64:128, :], b_d[:, :])
                bias_arg = bcol[:, 0:1]
            else:
                nc.gpsimd.dma_start(
                    constp.tile([1, 1], F32, name="bjunk")[:], b_d[0:1, :]
                )
                bias_arg = 0.0

            # --- load all x chunks up front (sync HWDGE ring, unused else) ---
            x_chunks = []
            for u in range(n_tiles):
                n0 = u * tn
                x_ch = xinp.tile(
                    [NPART, rpp, WDIM], F32, name=f"xch{u}", tag=f"xch{u}"
                )
                nc.sync.dma_start(
                    x_ch[:],
                    x_d[n0 : n0 + tn, :].rearrange("(q c) w -> q c w", c=rpp),
                )
                x_chunks.append(x_ch)

            prev_op = {}

            def chain(key, bi):
                if key in prev_op:
                    add_dep_helper(
                        bi.ins, prev_op[key].ins, sync=False, reason="tile op order"
                    )
                prev_op[key] = bi

            for u in [t for _ in range(repeat) for t in range(n_tiles)]:
                n0 = u * tn
                out_r = o_d[:, n0 : n0 + tn, :].rearrange(
                    "f (h j) p -> h f j p", h=2, j=half
                )

                # transpose row-groups: xt[w, c, j] = x[n0 + j*rpp + c, w]
                xt_ps = ps_xt.tile([64, rpp, NPART], F32, tag="xtp")
                for c in range(rpp):
                    chain(
                        "pe",
                        nc.tensor.transpose(
                            xt_ps[:, c, :], x_chunks[u][:, c, :], ident[:]
                        ),
                    )
                xt_sb = workp.tile([NPART, rpp, 64], F32, tag="xt_sb")
                chain("v", nc.vector.tensor_copy(xt_sb[0:64], xt_ps[:, :, 0:64]))
                chain("v", nc.vector.tensor_copy(xt_sb[64:128], xt_ps[:, :, 64:128]))

                # s broadcast over (h, f): s_ps[(h,f), m] = s[n0 + (tn/2)h + m]
                s_ps = ps_s.tile([NPART, half], F32, tag="s_ps")
                chain(
                    "pe",
                    nc.tensor.matmul(
                        s_ps[:], e_mat[:], xt_sb[:].rearrange("k c j -> k j c")
                    ),
                )

                out_t = outp.tile([NPART, half, P], odt, tag="out_t")
                for s in range(n_sub):
                    j0 = s * js
                    if evict == "none":
                        chain(
                            "v",
                            nc.vector.scalar_tensor_tensor(
                                out_t[:, j0 : j0 + js, :],
                                s_ps[:, j0 : j0 + js].to_broadcast((NPART, js, P)),
                                1.0,
                                wbig[:],
                                mybir.AluOpType.bypass,
                                mybir.AluOpType.mult,
                            ),
                        )
                        continue
                    t_ps = ps_t.tile([NPART, js, P], F32, tag="t_ps")
                    chain(
                        "v",
                        nc.vector.scalar_tensor_tensor(
                            t_ps[:],
                            s_ps[:, j0 : j0 + js].to_broadcast((NPART, js, P)),
                            1.0,
                            wbig[:],
                            mybir.AluOpType.bypass,
                            mybir.AluOpType.mult,
                        ),
                    )
                    use_act = (
                        evict == "act"
                        or (evict == "split" and s % 2 == 0)
                        or (evict == "split4" and s % 4 != 3)
                        or (evict == "split3" and s % 3 != 2)
                        or evict == "acts"
                    )
                    if use_act:
                        if evict == "acts":
                            bi = nc.scalar.tensor_scalar(
                                out_t[:, j0 : j0 + js, :],
                                t_ps[:],
                                0.0,
                                None,
                                mybir.AluOpType.max,
                            )
                        else:
                            bi = nc.scalar.activation(
                                out_t[:, j0 : j0 + js, :],
                                t_ps[:],
                                mybir.ActivationFunctionType.Relu,
                                bias=bias_arg,
                            )
                        chain("s", bi)
                    else:
                        chain(
                            "v",
                            nc.vector.tensor_scalar(
                                out_t[:, j0 : j0 + js, :],
                                t_ps[:],
                                0.0,
                                None,
                                mybir.AluOpType.max,
                            ),
                        )

                if not skip_out_dma:
                    nc.gpsimd.dma_start(out_r, out_t[:])

    nc.compile()
    return nc


def build_bass_v3(
    with_bias: bool,
    repeat: int = 1,
    skip_out_dma: bool = False,
    tile_sizes: tuple[int, ...] = (256, 256, 512, 512, 512, 512, 512, 512, 512),
    wbig_bcast: bool = True,
    relu_split: int = 4,   # stt/relu subs per tile (production granularity)
    dma_split: int = 4,    # output DMAs per tile (measured: fine > coarse in-kernel)
    js_floor: int = 64,    # minimum j extent per sub
    out_bufs: int = 3,
    inplace_relu: bool = True,
    out_dt=None,
) -> bass.Bass:
    """v3: minimal-op pipeline — per 64-row sub-tile one DVE stt produces
    the (128, js, 25) fp16 product in SBUF (in0 = s straight from PSUM
    broadcast over p, in1 = the (128, 25) fp16 W tile broadcast over j via
    a stride-0 AP), ScalarE applies relu (writing the DMA-facing out tile),
    and SWDGE cast-DMAs write f32 HBM in dma_split groups per tile.  The
    PSUM round-trip for the product and the per-sub DVE/ACT ping-pong of
    v2 are gone, so the DVE chain (~7.6 us/512-tile modeled, faster on HW)
    stays under the 9.1 us/tile DMA transfer and the SWDGE queue never
    starves.  Measured repeat-marginal device time ~60-77 us/iter (v2
    baseline: 131 us; pure-DMA floor at this granularity: ~53-73 us)."""
    nc = bacc.Bacc(None)

    x_d = nc.dram_tensor("x", (N_LOCAL, WDIM), F32, kind="ExternalInput")
    w_d = nc.dram_tensor("W", (F, P), F32, kind="ExternalInput")
    b_d = nc.dram_tensor("b", (F, 1), F32, kind="ExternalInput")
    o_d = nc.dram_tensor("out", (F, N_LOCAL, P), F32, kind="ExternalOutput")

    assert sum(tile_sizes) == N_LOCAL
    odt = out_dt if out_dt is not None else F16
    max_half = max(tile_sizes) // 2

    with tile.TileContext(nc) as tc:
        with (
            tc.tile_pool(name="const", bufs=1) as constp,
            tc.tile_pool(name="xin", bufs=1) as xinp,
            tc.tile_pool(name="work", bufs=2) as workp,
            tc.tile_pool(name="outp", bufs=out_bufs) as outp,
            tc.tile_pool(name="prodp", bufs=2) as prodp,
            tc.tile_pool(name="ps_xt", bufs=2, space="PSUM") as ps_xt,
            tc.tile_pool(name="ps_s", bufs=2, space="PSUM") as ps_s,
        ):
            ident = constp.tile([NPART, NPART], F32)
            masks.make_identity(nc, ident[:])

            e_mat = constp.tile([NPART, NPART], F32)
            nc.gpsimd.memset(e_mat[:], 0.0)
            nc.gpsimd.memset(e_mat[0:64, 0:64], 1.0)
            nc.gpsimd.memset(e_mat[64:128, 64:128], 1.0)

            # fp16 W columns on both partition halves (cast on load)
            wc16 = constp.tile([NPART, P], F16)
            nc.gpsimd.dma_start(wc16[0:64, :], w_d[:, :])
            nc.gpsimd.dma_start(wc16[64:128, :], w_d[:, :])
            if wbig_bcast:
                w_in1 = wc16[:].rearrange("q (j p) -> q j p", j=1)
            else:
                wbig = constp.tile([NPART, max_half, P], F16)
                nc.vector.tensor_copy(
                    wbig[:],
                    wc16[:].rearrange("q (o p) -> q o p", o=1).to_broadcast(
                        (NPART, max_half, P)
                    ),
                )
            # ACT table warm-up: the per-tile relu runs on ScalarE
            warm = constp.tile([NPART, 1], F32)
            nc.vector.memset(warm[:], 0.0)
            warm_out = constp.tile([NPART, 1], F32)
            nc.scalar.activation(
                warm_out[:], warm[:], mybir.ActivationFunctionType.Relu
            )
            if with_bias:
                bcol = constp.tile([NPART, 1], F32)
                nc.scalar.dma_start(bcol[0:64, :], b_d[:, :])
                nc.scalar.dma_start(bcol[64:128, :], b_d[:, :])
                bias_arg = bcol[:, 0:1]
            else:
                nc.gpsimd.dma_start(
                    constp.tile([1, 1], F32, name="bjunk")[:], b_d[0:1, :]
                )
                bias_arg = 0.0

            tile_offsets = [sum(tile_sizes[:u]) for u in range(len(tile_sizes))]

            # --- load all x chunks up front (SP HWDGE ring, unused else) ---
            x_chunks = []
            for u, (n0, tn) in enumerate(zip(tile_offsets, tile_sizes)):
                rpp = tn // NPART
                x_ch = xinp.tile(
                    [NPART, rpp, WDIM], F32, name=f"xch{u}", tag=f"xch{u}"
                )
                nc.sync.dma_start(
                    x_ch[:],
                    x_d[n0 : n0 + tn, :].rearrange("(q c) w -> q c w", c=rpp),
                )
                x_chunks.append(x_ch)

            prev_op = {}

            def chain(key, bi):
                if key in prev_op:
                    add_dep_helper(
                        bi.ins, prev_op[key].ins, sync=False, reason="tile op order"
                    )
                prev_op[key] = bi

            # Software-pipelined: the s-pipeline (PE transposes -> ScalarE
            # column-split copies -> PE reduce matmul) for tile u+1 is
            # emitted BEFORE tile u's sub loop, so on every engine chain it
            # sits ahead of tile u's stt/relu work and the next tile's stt
            # can start the moment the previous tile's last stt retires.
            sched = [t for _ in range(repeat) for t in range(len(tile_sizes))]

            def s_stage(u):
                n0, tn = tile_offsets[u], tile_sizes[u]
                rpp = tn // NPART
                half = tn // 2
                xt_ps = ps_xt.tile([64, rpp, NPART], F32, tag="xtp")
                for c in range(rpp):
                    chain(
                        "pe",
                        nc.tensor.transpose(
                            xt_ps[:, c, :], x_chunks[u][:, c, :], ident[:]
                        ),
                    )
                xt_sb = workp.tile([NPART, rpp, 64], F32, tag="xt_sb")
                chain("s", nc.scalar.copy(xt_sb[0:64], xt_ps[:, :, 0:64]))
                chain("s", nc.scalar.copy(xt_sb[64:128], xt_ps[:, :, 64:128]))
                s_ps = ps_s.tile([NPART, half], F32, tag="s_ps")
                chain(
                    "pe",
                    nc.tensor.matmul(
                        s_ps[:], e_mat[:], xt_sb[:].rearrange("k c j -> k j c")
                    ),
                )
                return s_ps

            s_pipe = {0: s_stage(sched[0])}

            for w, u in enumerate(sched):
                n0, tn = tile_offsets[u], tile_sizes[u]
                half = tn // 2
                out_r = o_d[:, n0 : n0 + tn, :].rearrange(
                    "f (h j) p -> h f j p", h=2, j=half
                )
                if w + 1 < len(sched):
                    s_pipe[w + 1] = s_stage(sched[w + 1])
                s_ps = s_pipe.pop(w)

                # Per-sub production: DVE product -> ScalarE relu ->
                # SWDGE cast-DMA, fine-grained so the chains pipeline.  DMA
                # granularity is independent (dma_split); measured on HW the
                # in-kernel optimum is ~0.8 MB DMAs (dma_split=4 at tn=512):
                # coarser groups serialize production against the queue
                # (205us at dma_split=1), finer adds per-DMA overhead.
                out_t = outp.tile([NPART, half, P], odt, tag="out_t")
                js = max(half // max(relu_split, 1), js_floor)
                n_sub = max(half // js, 1)
                js = half // n_sub
                n_dma = min(max(dma_split, 1), n_sub)
                subs_per_dma = (n_sub + n_dma - 1) // n_dma
                for si in range(n_sub):
                    j0 = si * js
                    sl = out_t[:, j0 : j0 + js, :]
                    if inplace_relu:
                        prod_sl = sl
                    else:
                        prod_t = prodp.tile([NPART, js, P], odt, tag="prod_t")
                        prod_sl = prod_t[:]
                    in1 = (
                        w_in1.to_broadcast((NPART, js, P))
                        if wbig_bcast
                        else wbig[:, 0:js, :]
                    )
                    chain(
                        "v",
                        nc.vector.scalar_tensor_tensor(
                            prod_sl,
                            s_ps[:, j0 : j0 + js].to_broadcast((NPART, js, P)),
                            1.0,
                            in1,
                            mybir.AluOpType.bypass,
                            mybir.AluOpType.mult,
                        ),
                    )
                    chain(
                        "s",
                        nc.scalar.activation(
                            sl, prod_sl, mybir.ActivationFunctionType.Relu,
                            bias=bias_arg,
                        ),
                    )
                    if not skip_out_dma and (si + 1) % subs_per_dma == 0:
                        g0 = (si + 1 - subs_per_dma) * js
                        g1 = (si + 1) * js
                        nc.gpsimd.dma_start(
                            out_r[:, :, g0:g1, :], out_t[:, g0:g1, :]
                        )

    nc.compile()
    return nc


_CACHE: dict[bool, bass.Bass] = {}


# Production configuration (single source of truth for _get_bass and the
# timing harness's repeat builds).
PROD_CFG = dict(
    tile_sizes=(256, 256, 512, 512, 512, 512, 512, 512, 512),
    relu_split=4,
    dma_split=4,
    out_bufs=4,
    inplace_relu=False,
)


def _build_timing(with_bias: bool, repeat: int = 1) -> bass.Bass:
    """Production kernel with the per-tile pipeline repeated `repeat` times
    (identical work each repeat) — lets the timing harness cancel the
    per-dispatch proxy overhead via a repeat-marginal measurement."""
    return build_bass_v3(with_bias, repeat=repeat, **PROD_CFG)


def _get_bass(with_bias: bool) -> bass.Bass:
    if with_bias not in _CACHE:
        _CACHE[with_bias] = build_bass_v3(with_bias, **PROD_CFG)
    return _CACHE[with_bias]


last_exec_time_ns = None
last_profile = None


def kernel(x, W, b, trace=False, **run_kwargs):
    global last_exec_time_ns, last_profile
    x = np.ascontiguousarray(np.asarray(x, dtype=np.float32)).reshape(N_TOTAL, WDIM)
    wf = np.ascontiguousarray(np.asarray(W, dtype=np.float32)).reshape(F, P)
    bf = np.ascontiguousarray(np.asarray(b, dtype=np.float32)).reshape(F, 1)

    nc = _get_bass(bool(np.any(bf)))

    in_maps = [
        {
            "x": x[m * N_LOCAL : (m + 1) * N_LOCAL],
            "W": wf,
            "b": bf,
        }
        for m in range(N_CORES)
    ]
    res = run_bass_kernel_spmd(
        nc, in_maps, core_ids=list(range(N_CORES)), trace=trace, **run_kwargs
    )
    last_exec_time_ns = res.exec_time_ns
    last_profile = res.profile_json
    outs = [np.asarray(res.results[m]["out"]) for m in range(N_CORES)]
    full = np.concatenate(outs, axis=1)  # (F, N_TOTAL, P)
    return full[None]

